# revision 1
# baseline (speedup 1.0000x reference)
"""AttentionPool Trainium2 Bass kernel.

Computes, for h:[N,512] f32, sorted batch_vec:[N] int, gate-MLP weights
W1/b1/W2/b2:
    gate  = gelu(h @ W1 + b1) @ W2 + b2            (erf gelu)
    alpha = segment_softmax(gate, batch_vec)       (1024 segments)
    out   = segment_sum(alpha[:,None] * h)         -> [1024, 512]

Sharding: data-parallel over graphs. Core c owns graphs [128c, 128c+128)
and the contiguous node range covering them (batch_vec sorted => segments
never straddle cores). Each core gets its padded node shard in BOTH
layouts (h and h^T, prepared on host), computes gates with the tensor
engine (z^T = W1^T @ h^T so no on-chip transposes are needed), applies
exp once over the compact gate vector (single ACT table set switch), and
pools via matmul against a one-hot selection matrix built on-chip from
batch_vec with a fused (iota == bv) * e vector op:
    pooled[g,:] += sum_n sel_w[n,g] * h[n,:]   (PSUM accumulation)
    denom[g]    += sum_n sel_w[n,g]
    out = pooled / denom
The softmax max-subtraction is skipped: gates are O(1) so exp is safe in
fp32, and the result is mathematically identical.
"""

import os
from contextlib import ExitStack

import numpy as np

import concourse.bass as bass
import concourse.mybir as mybir
from concourse import bacc
import concourse.tile as tile
from concourse.bass_utils import run_bass_kernel_spmd

F32 = mybir.dt.float32
BF16 = mybir.dt.bfloat16

N_NODES = 100000
H = 512
NUM_GRAPHS = 1024
N_CORES = 8
G = NUM_GRAPHS // N_CORES  # graphs per core = 128
NP_DEFAULT = 12800         # padded nodes per core (25 supertiles of 512)

_DT = {"f32": F32, "bf16": BF16, "f32r": mybir.dt.float32r,
       "f16": mybir.dt.float16}
_NPDT = {"f32": np.float32, "f32r": np.float32, "f16": np.float16}
try:
    import ml_dtypes
    _NPDT["bf16"] = ml_dtypes.bfloat16
except ImportError:
    pass

DT_GATE = os.environ.get("AP_DT_GATE", "f16")   # hT / W1 matmul dtype
DT_POOL = os.environ.get("AP_DT_POOL", "f16")   # h-plain / selection dtype
GELU = os.environ.get("AP_GELU", "gelu")        # "gelu" | "tanh" (sim only)
DT_A1 = os.environ.get("AP_DT_A1", "f16")       # a1 / W2 dtype (gate matmul)
HP_BUFS = int(os.environ.get("AP_HP_BUFS", "14"))
HT_BUFS = int(os.environ.get("AP_HT_BUFS", "4"))


def _build(np_pad: int, dt_gate: mybir.dt, dt_pool: mybir.dt, reps: int = 1,
           ablate: str = "", dt_a1: mybir.dt | None = None):
    if dt_a1 is None:
        dt_a1 = _DT[DT_A1]
    """Build the per-core Bass program (SPMD: same program, per-core data)."""
    T = np_pad // 128          # 128-node tiles
    S = np_pad // 512          # 512-node supertiles
    KC = H // 128              # contraction chunks = 4

    nc = bacc.Bacc("TRN2", target_bir_lowering=False, debug=False)

    ht_d = nc.dram_tensor("hT", [H, np_pad], dt_gate, kind="ExternalInput")
    hp_d = nc.dram_tensor("hp", [np_pad, H], dt_pool, kind="ExternalInput")
    w1_d = nc.dram_tensor("W1", [H, H], dt_gate, kind="ExternalInput")
    b1_d = nc.dram_tensor("b1v", [128, KC], F32, kind="ExternalInput")
    w2_d = nc.dram_tensor("W2v", [128, KC * 2], dt_a1, kind="ExternalInput")
    b2_d = nc.dram_tensor("b2t", [128, 1], F32, kind="ExternalInput")
    bv_d = nc.dram_tensor("bvrel", [128, T], F32, kind="ExternalInput")
    io_d = nc.dram_tensor("iota", [128, 128], F32, kind="ExternalInput")
    out_d = nc.dram_tensor("out", [G, H], F32, kind="ExternalOutput")

    gelu_func = (mybir.ActivationFunctionType.Gelu if GELU == "gelu"
                 else mybir.ActivationFunctionType.Tanh)

    with tile.TileContext(nc) as tc, ExitStack() as ctx:
        consts = ctx.enter_context(tc.tile_pool(name="consts", bufs=1))
        ht_pool = ctx.enter_context(tc.tile_pool(name="ht", bufs=HT_BUFS))
        a1_pool = ctx.enter_context(tc.tile_pool(name="a1", bufs=8))
        hp_pool = ctx.enter_context(tc.tile_pool(name="hp", bufs=HP_BUFS))
        ms_pool = ctx.enter_context(tc.tile_pool(name="ms", bufs=4))
        small = ctx.enter_context(tc.tile_pool(name="small", bufs=2))
        psz = ctx.enter_context(tc.tile_pool(name="psz", bufs=4, space="PSUM"))
        psg = ctx.enter_context(tc.tile_pool(name="psg", bufs=2, space="PSUM"))
        psp = ctx.enter_context(tc.tile_pool(name="psp", bufs=1, space="PSUM"))
        psd = ctx.enter_context(tc.tile_pool(name="psd", bufs=1, space="PSUM"))

        w1_sb = []
        for k in range(KC):
            t = consts.tile([128, H], dt_gate, tag=f"w1_{k}")
            nc.sync.dma_start(out=t, in_=w1_d.ap()[k * 128:(k + 1) * 128, :])
            w1_sb.append(t)
        b1_sb = consts.tile([128, KC], F32, tag="b1")
        nc.sync.dma_start(out=b1_sb, in_=b1_d.ap())
        w2_sb = consts.tile([128, KC * 2], dt_a1, tag="w2")
        nc.sync.dma_start(out=w2_sb, in_=w2_d.ap())
        b2_sb = consts.tile([128, 1], F32, tag="b2")
        nc.sync.dma_start(out=b2_sb, in_=b2_d.ap())
        io_sb = consts.tile([128, 128], F32, tag="iota")
        nc.sync.dma_start(out=io_sb, in_=io_d.ap())
        bv_sb = consts.tile([128, T], F32, tag="bv")
        nc.sync.dma_start(out=bv_sb, in_=bv_d.ap())
        ones_dt = F32 if dt_pool == mybir.dt.float32r else dt_pool
        ones_sb = consts.tile([128, 2], ones_dt, tag="ones")
        nc.vector.memset(ones_sb, 1.0)
        ones_mm = (ones_sb[:, 0:2].bitcast(mybir.dt.float32r)
                   if dt_pool == mybir.dt.float32r else ones_sb[:, 0:2])
        gate_sb = consts.tile([128, T], F32, tag="gate")
        e_sb = consts.tile([128, T], F32, tag="e")

        from contextlib import nullcontext
        loop_cm = tc.For_i(0, reps, 1) if reps > 1 else nullcontext()
        with loop_cm:
            # ---- Phase A: gate^T via z^T = W1^T @ h^T (all gelus batched) ----
            if ablate == "noA":
                nc.vector.memset(gate_sb, 0.125)
            ht4 = ht_d.ap().rearrange("(k p) (s n) -> s p k n", p=128, n=512)
            for s in (range(S) if ablate != "noA" else []):
                htb = ht_pool.tile([128, KC, 512], dt_gate, tag="ht")
                nc.sync.dma_start(out=htb, in_=ht4[s])
                hts = [htb[:, k, :] for k in range(KC)]
                if ablate == "dmaonly":
                    continue
                a1s = []
                for d in range(KC):
                    pz = psz.tile([128, 512], F32, tag="pz")
                    for k in range(KC):
                        nc.tensor.matmul(
                            out=pz,
                            lhsT=w1_sb[k][:, d * 128:(d + 1) * 128],
                            rhs=hts[k],
                            start=(k == 0), stop=(k == KC - 1))
                    a1 = a1_pool.tile([128, 512], dt_a1, tag="a1")
                    nc.scalar.activation(out=a1, in_=pz, func=gelu_func,
                                         bias=b1_sb[:, d:d + 1], scale=1.0)
                    a1s.append(a1)
                if ablate == "nogate":
                    nc.vector.memset(gate_sb[:, s * 4:(s + 1) * 4], 0.125)
                    continue
                pg = psg.tile([128, 2 * KC], F32, tag="pg")
                for nch in range(4):
                    for d in range(KC):
                        nc.tensor.matmul(
                            out=pg[:, 2 * nch:2 * nch + 2],
                            lhsT=a1s[d][:, nch * 128:(nch + 1) * 128],
                            rhs=w2_sb[:, 2 * d:2 * d + 2],
                            start=(d == 0), stop=(d == KC - 1))
                nc.vector.tensor_copy(out=gate_sb[:, s * 4:(s + 1) * 4],
                                      in_=pg[:, 0:2 * KC:2])
                if ablate == "" and s == S // 2 - 1:
                    t_mid = (s + 1) * 4
                    nc.scalar.activation(
                        out=e_sb[:, 0:t_mid], in_=gate_sb[:, 0:t_mid],
                        func=mybir.ActivationFunctionType.Exp,
                        bias=b2_sb[:, 0:1], scale=1.0)
            if ablate == "dmaonly":
                nc.vector.memset(gate_sb, 0.125)

            # ---- Phase B: e = exp(gate + b2) (tail half if split) ----
            t_mid = (S // 2) * 4 if ablate == "" else 0
            nc.scalar.activation(out=e_sb[:, t_mid:T], in_=gate_sb[:, t_mid:T],
                                 func=mybir.ActivationFunctionType.Exp,
                                 bias=b2_sb[:, 0:1], scale=1.0)

            # ---- Phase C: pooled = sel_w^T @ h, denom = sel_w^T @ 1 ----
            if ablate not in ("noC", "dmaonly"):
                pp = psp.tile([128, H], F32, tag="pp")
                pd = psd.tile([128, 2], F32, tag="pd")
            hp4 = hp_d.ap().rearrange("(s j p) d -> s p j d", p=128, j=4)
            for s4 in (range(S) if ablate not in ("noC",) else []):
                hpb = hp_pool.tile([128, 4, H], dt_pool, tag="hp")
                nc.sync.dma_start(out=hpb, in_=hp4[s4])
                if ablate == "dmaonly":
                    continue
              # per-128 tiles within the batched load
                for j in range(4):
                    t = s4 * 4 + j
                    ms = ms_pool.tile([128, 128], dt_pool, tag="ms")
                    # ms[n, g] = (iota[n,g] == bvrel[n]) * e[n]
                    nc.vector.tensor_scalar(
                        out=ms, in0=io_sb,
                        scalar1=bv_sb[:, t:t + 1], scalar2=e_sb[:, t:t + 1],
                        op0=mybir.AluOpType.is_equal, op1=mybir.AluOpType.mult)
                    nc.tensor.matmul(out=pp, lhsT=ms, rhs=hpb[:, j, :],
                                     start=(t == 0), stop=(t == T - 1))
                    nc.tensor.matmul(out=pd, lhsT=ms, rhs=ones_mm,
                                     start=(t == 0), stop=(t == T - 1))

            osb = small.tile([128, H], F32, tag="osb")
            if ablate in ("noC", "dmaonly"):
                nc.vector.memset(osb, 0.0)
            else:
                dcl = small.tile([128, 1], F32, tag="dcl")
                nc.vector.tensor_scalar(out=dcl, in0=pd[:, 0:1], scalar1=1e-35,
                                        scalar2=None, op0=mybir.AluOpType.max)
                rec = small.tile([128, 1], F32, tag="rec")
                nc.vector.reciprocal(out=rec, in_=dcl)
                nc.vector.tensor_scalar(out=osb, in0=pp, scalar1=rec[:, 0:1],
                                        scalar2=None, op0=mybir.AluOpType.mult)
            nc.sync.dma_start(out=out_d.ap(), in_=osb)

    nc.compile()
    return nc


_prog_cache: dict = {}


def _get_prog(np_pad: int):
    key = (np_pad, DT_GATE, DT_POOL, DT_A1, GELU)
    if key not in _prog_cache:
        _prog_cache[key] = _build(np_pad, _DT[DT_GATE], _DT[DT_POOL])
    return _prog_cache[key]


def _prep_in_maps(h, bv, W1, b1, W2, b2, np_pad):
    """Shard + pad inputs per core; returns list of per-core input dicts."""
    npdt_g = _NPDT[DT_GATE]
    npdt_p = _NPDT[DT_POOL]
    T = np_pad // 128
    bounds = np.searchsorted(bv, np.arange(0, NUM_GRAPHS + 1, G))

    w1_full = np.ascontiguousarray(W1.astype(npdt_g))
    b1v = np.ascontiguousarray(b1.astype(np.float32).reshape(4, 128).T)
    npdt_a1 = _NPDT[DT_A1]
    w2v = np.zeros((128, 8), npdt_a1)
    w2v[:, 0::2] = W2[:, 0].astype(npdt_a1).reshape(4, 128).T
    b2t = np.full((128, 1), np.float32(b2.reshape(-1)[0]), np.float32)
    iota = np.ascontiguousarray(
        np.tile(np.arange(128, dtype=np.float32), (128, 1)))

    in_maps = []
    for c in range(N_CORES):
        n0, n1 = int(bounds[c]), int(bounds[c + 1])
        cnt = n1 - n0
        hp = np.zeros((np_pad, H), np.float32)
        hp[:cnt] = h[n0:n1]
        ht = np.zeros((H, np_pad), np.float32)
        ht[:, :cnt] = h[n0:n1].T
        bvrel = np.full(np_pad, -1.0, np.float32)
        bvrel[:cnt] = bv[n0:n1].astype(np.float32) - c * G
        bvrel = np.ascontiguousarray(bvrel.reshape(T, 128).T)
        in_maps.append({
            "hT": np.ascontiguousarray(ht.astype(npdt_g)),
            "hp": np.ascontiguousarray(hp.astype(npdt_p)),
            "W1": w1_full,
            "b1v": b1v,
            "W2v": w2v,
            "b2t": b2t,
            "bvrel": bvrel,
            "iota": iota,
        })
    return in_maps


def kernel(**inputs) -> np.ndarray:
    h = np.ascontiguousarray(np.asarray(inputs["h"], dtype=np.float32))
    bv = np.asarray(inputs["batch_vec"]).astype(np.int64)
    W1 = np.asarray(inputs["W1"], dtype=np.float32)
    b1 = np.asarray(inputs["b1"], dtype=np.float32)
    W2 = np.asarray(inputs["W2"], dtype=np.float32)
    b2 = np.asarray(inputs["b2"], dtype=np.float32)

    bounds = np.searchsorted(bv, np.arange(0, NUM_GRAPHS + 1, G))
    max_cnt = int(np.diff(bounds).max())
    np_pad = NP_DEFAULT
    if max_cnt > np_pad:  # fallback for unexpected distributions
        np_pad = ((max_cnt + 511) // 512) * 512

    nc = _get_prog(np_pad)
    in_maps = _prep_in_maps(h, bv, W1, b1, W2, b2, np_pad)
    trace = bool(int(os.environ.get("AP_TRACE", "0")))
    res = run_bass_kernel_spmd(nc, in_maps, list(range(N_CORES)), trace=trace)
    global last_results
    last_results = res
    out = np.concatenate([res.results[c]["out"] for c in range(N_CORES)],
                         axis=0).astype(np.float32)
    return out


last_results = None

